# revision 13
# baseline (speedup 1.0000x reference)
"""Trainium2 Bass kernel for a GRU decoder with Luong attention.

Problem (hardcoded shapes): B=32, S=64, T=64, H=512, V=32000.
  out = log_softmax(decoder(inputs)) with shape [B, T, V] fp32.

Sharding: data-parallel over batch. Each of the 8 cores processes 4 batch
rows end-to-end (embedding gather, GRU recurrence, Luong attention, output
projection, local log-softmax over the full vocab). No collectives.

Per-core row order for the 256 output rows is t-major: r = t*4 + b_local.
"""

import numpy as np
import ml_dtypes

import concourse.bacc as bacc
import concourse.bass as bass
import concourse.mybir as mybir
import concourse.tile as tile
from concourse.masks import make_identity

F32 = mybir.dt.float32
BF16 = mybir.dt.bfloat16
I32 = mybir.dt.int32
AF = mybir.ActivationFunctionType
ALU = mybir.AluOpType
AX = mybir.AxisListType
F32R = mybir.dt.float32r


def rr(ap):
    return ap.bitcast(F32R)

B, S, T, H, V = 32, 64, 64, 512, 32000
NC = 8
BL = B // NC          # 4 local batch rows
R = T * BL            # 256 local output rows, r = t*BL + b
VCHUNK = 500          # vocab chunk for the output matmul (<=512, divides V)
NVCH = V // VCHUNK    # 64
OCHUNK = 2000         # output store chunk
NEG = -1e30


def build_program(dbg=False):
    nc = bacc.Bacc(None, target_bir_lowering=False, debug=False)

    # ---- DRAM parameters (per-core slices prepared on host) ----
    emb_d = nc.declare_dram_parameter("emb", [V, H], F32, isOutput=False)
    ids_d = nc.declare_dram_parameter("ids", [2, 128, 1], I32, isOutput=False)
    h0_d = nc.declare_dram_parameter("h0", [BL, H], F32, isOutput=False)
    encT_d = nc.declare_dram_parameter("encT", [H, BL * S], F32, isOutput=False)
    encS_d = nc.declare_dram_parameter("encS", [S, BL * H], F32, isOutput=False)
    maskb_d = nc.declare_dram_parameter("maskb", [1, BL * S], F32, isOutput=False)
    actm_d = nc.declare_dram_parameter("actm", [BL, T], F32, isOutput=False)
    wihT_d = nc.declare_dram_parameter("wihT", [H, 3 * H], F32, isOutput=False)
    whhT_d = nc.declare_dram_parameter("whhT", [H, 3 * H], F32, isOutput=False)
    bihh_d = nc.declare_dram_parameter("bihh", [1, 3 * H], F32, isOutput=False)
    wccT_d = nc.declare_dram_parameter("wccT", [2 * H, H], F32, isOutput=False)
    bcc_d = nc.declare_dram_parameter("bcc", [128, 4], F32, isOutput=False)
    woT_d = nc.declare_dram_parameter("woT", [H, V], BF16, isOutput=False)
    bout_d = nc.declare_dram_parameter("bout", [1, V], BF16, isOutput=False)
    ones_d = nc.declare_dram_parameter("onesd", [1, 128], F32, isOutput=False)
    out_d = nc.declare_dram_parameter("out", [R, V], F32, isOutput=True)

    gx_d = nc.dram_tensor("gx_stage", [R, 3 * H], F32)
    if dbg:
        dbg_hnewT = nc.declare_dram_parameter("dbg_hnewT", [128, T * 16], F32, isOutput=True)
        dbg_ctxT = nc.declare_dram_parameter("dbg_ctxT", [128, T * 16], F32, isOutput=True)
        dbg_hot = nc.declare_dram_parameter("dbg_hot", [4, 128, R], F32, isOutput=True)

    with tile.TileContext(nc) as tc:
      with (
        tc.tile_pool(name="const", bufs=1) as constp,
        tc.tile_pool(name="hist", bufs=1) as histp,
        tc.tile_pool(name="hot", bufs=1) as hotp,
        tc.tile_pool(name="wo", bufs=8) as wop,
        tc.tile_pool(name="bo", bufs=4) as bop,
      ):
        # ---- constants ----
        ident = constp.tile([128, 128], F32, tag="ident")
        make_identity(nc, ident[:])
        ones_f = constp.tile([1, 128], F32, tag="ones_f")
        nc.sync.dma_start(rr(ones_f[:]), rr(ones_d[:]))
        ones_b = constp.tile([1, 128], BF16, tag="ones_b")
        nc.vector.memset(ones_b[:], 1.0)
        maskb = constp.tile([1, BL * S], F32, tag="maskb")
        nc.sync.dma_start(rr(maskb[:]), rr(maskb_d[:]))
        actm = constp.tile([BL, T], F32, tag="actm")
        nc.sync.dma_start(actm[:], actm_d[:])
        bcc = constp.tile([128, 4], F32, tag="bcc")
        nc.sync.dma_start(bcc[:], bcc_d[:])

        # history buffers: col = t*16 + q*4 + b for k-tile q, step t
        hnewT = histp.tile([128, T * 16], F32, tag="hnewT")
        ctxT = histp.tile([128, T * 16], F32, tag="ctxT")
        hot = [hotp.tile([128, R], BF16, tag=f"hot{mh}", name=f"hot{mh}") for mh in range(4)]

        with (
            tc.tile_pool(name="weights", bufs=1) as wp,
            tc.tile_pool(name="xs", bufs=2) as xsp,
            tc.tile_pool(name="xsT", bufs=8) as xstp,
            tc.tile_pool(name="gxsb", bufs=3) as gxsbp,
            tc.tile_pool(name="gxt", bufs=3) as gxtp,
            tc.tile_pool(name="hT", bufs=2) as hTp,
            tc.tile_pool(name="gates", bufs=2) as gp,
            tc.tile_pool(name="h", bufs=3) as hp,
            tc.tile_pool(name="att", bufs=3) as attp,
            tc.tile_pool(name="ps_a", bufs=2, space="PSUM") as ps_a,
            tc.tile_pool(name="ps_gh", bufs=5, space="PSUM") as ps_gh,
        ):
            bihh = wp.tile([1, 3 * H], F32, tag="bihh")
            nc.sync.dma_start(rr(bihh[:]), rr(bihh_d[:]))
            wih, whh = [], []
            for q in range(4):
                wt = wp.tile([128, 3 * H], F32, tag=f"wih{q}")
                nc.sync.dma_start(rr(wt[:]), rr(wihT_d[q * 128:(q + 1) * 128, :]))
                wih.append(wt)
                ht = wp.tile([128, 3 * H], F32, tag=f"whh{q}")
                nc.sync.dma_start(rr(ht[:]), rr(whhT_d[q * 128:(q + 1) * 128, :]))
                whh.append(ht)
            wcc = []
            for kt in range(8):
                w = wp.tile([128, H], F32, tag=f"wcc{kt}")
                nc.sync.dma_start(rr(w[:]), rr(wccT_d[kt * 128:(kt + 1) * 128, :]))
                wcc.append(w)
            encT = []
            for q in range(4):
                e = wp.tile([128, BL * S], F32, tag=f"encT{q}")
                nc.sync.dma_start(rr(e[:]), rr(encT_d[q * 128:(q + 1) * 128, :]))
                encT.append(e)
            encS = wp.tile([S, BL * H], F32, tag="encS")
            nc.sync.dma_start(rr(encS[:]), rr(encS_d[:]))

            # ---- P1: embedding gather + bulk gx = xs @ W_ih.T + (b_ih+b_hh) ----
            for m in range(2):
                ids_t = xsp.tile([128, 1], I32, tag="ids")
                nc.sync.dma_start(ids_t[:], ids_d[m])
                xs_t = xsp.tile([128, H], F32, tag="xs")
                nc.gpsimd.indirect_dma_start(
                    out=xs_t[:],
                    out_offset=None,
                    in_=emb_d[:],
                    in_offset=bass.IndirectOffsetOnAxis(ap=ids_t[:, 0:1], axis=0),
                )
                xsT = []
                for q in range(4):
                    tp = ps_a.tile([128, 128], F32, tag="A")
                    nc.tensor.transpose(tp[:], xs_t[:, q * 128:(q + 1) * 128], ident[:])
                    xt = xstp.tile([128, 128], F32, tag="xsT")
                    nc.vector.tensor_copy(rr(xt[:]), tp[:])
                    xsT.append(xt)
                for j in range(3):
                    ps = ps_gh.tile([128, 512], F32, tag="GH")
                    for q in range(4):
                        nc.tensor.matmul(
                            ps[:], rr(xsT[q][:]), rr(wih[q][:, j * 512:(j + 1) * 512]),
                            start=(q == 0), stop=False,
                        )
                    nc.tensor.matmul(
                        ps[:], rr(ones_f[0:1, :]), rr(bihh[0:1, j * 512:(j + 1) * 512]),
                        start=False, stop=True,
                    )
                    gsb = gxsbp.tile([128, 512], F32, tag="gxsb")
                    nc.scalar.copy(gsb[:], ps[:])
                    nc.sync.dma_start(
                        gx_d[m * 128:(m + 1) * 128, j * 512:(j + 1) * 512], gsb[:]
                    )

            # ---- P2: GRU recurrence over T steps ----
            h_cur = hp.tile([BL, H], F32, tag="h")
            nc.sync.dma_start(h_cur[:], h0_d[:])

            for t in range(T):
                gxt = gxtp.tile([BL, 3 * H], F32, tag="gxt")
                nc.sync.dma_start(gxt[:], gx_d[t * BL:(t + 1) * BL, :])

                tp = ps_a.tile([128, 16], F32, tag="A")
                for q in range(4):
                    nc.tensor.transpose(
                        tp[:, q * 4:(q + 1) * 4],
                        h_cur[:, q * 128:(q + 1) * 128],
                        ident[0:BL, 0:BL],
                    )
                hT = hTp.tile([128, 16], F32, tag="hT")
                nc.vector.tensor_copy(rr(hT[:]), tp[:])

                ghs = []
                for j in range(3):
                    gh_j = ps_gh.tile([BL, H], F32, tag="GH", name=f"gh{t}_{j}")
                    for q in range(4):
                        nc.tensor.matmul(
                            gh_j[:],
                            rr(hT[:, q * 4:(q + 1) * 4]),
                            rr(whh[q][:, j * 512:(j + 1) * 512]),
                            start=(q == 0), stop=(q == 3),
                        )
                    ghs.append(gh_j)
                ghr, ghz, ghn = ghs

                rpre = gp.tile([BL, H], F32, tag="rpre")
                nc.vector.tensor_tensor(rpre[:], gxt[:, 0:H], ghr[:], ALU.add)
                r_ = gp.tile([BL, H], F32, tag="r")
                nc.scalar.activation(r_[:], rpre[:], AF.Sigmoid)
                zpre = gp.tile([BL, H], F32, tag="zpre")
                nc.vector.tensor_tensor(zpre[:], gxt[:, H:2 * H], ghz[:], ALU.add)
                u_ = gp.tile([BL, H], F32, tag="u")
                nc.scalar.activation(u_[:], zpre[:], AF.Sigmoid, scale=-1.0)
                t1 = gp.tile([BL, H], F32, tag="t1")
                nc.vector.tensor_tensor(t1[:], r_[:], ghn[:], ALU.mult)
                npre = gp.tile([BL, H], F32, tag="npre")
                nc.vector.tensor_tensor(npre[:], t1[:], gxt[:, 2 * H:3 * H], ALU.add)
                n_ = gp.tile([BL, H], F32, tag="n")
                nc.scalar.activation(n_[:], npre[:], AF.Tanh)
                d_ = gp.tile([BL, H], F32, tag="d")
                nc.vector.tensor_tensor(d_[:], n_[:], h_cur[:], ALU.subtract)
                w_ = gp.tile([BL, H], F32, tag="w")
                nc.vector.tensor_tensor(w_[:], u_[:], d_[:], ALU.mult)
                hnxt = hp.tile([BL, H], F32, tag="h")
                nc.vector.scalar_tensor_tensor(
                    hnxt[:], w_[:], actm[:, t:t + 1], h_cur[:],
                    op0=ALU.mult, op1=ALU.add,
                )

                tp2 = ps_a.tile([128, 16], F32, tag="A")
                for q in range(4):
                    nc.tensor.transpose(
                        tp2[:, q * 4:(q + 1) * 4],
                        w_[:, q * 128:(q + 1) * 128],
                        ident[0:BL, 0:BL],
                    )
                # h_new = h + w  =>  hnewT col = hT + wT
                nc.vector.tensor_tensor(
                    rr(hnewT[:, t * 16:(t + 1) * 16]), hT[:], tp2[:], ALU.add
                )

                h_cur = hnxt

                if t % 16 == 15:
                    blk = t // 16
                    c0, c1 = blk * 256, (blk + 1) * 256
                    for b in range(BL):
                        sc = ps_a.tile([16, S], F32, tag="A", name=f"sc{blk}_{b}")
                        for q in range(4):
                            nc.tensor.matmul(
                                sc[:],
                                rr(hnewT[:, c0 + q * 4 + b:c1:16]),
                                rr(encT[q][:, b * S:(b + 1) * S]),
                                start=(q == 0), stop=False,
                            )
                        nc.tensor.matmul(
                            sc[:], rr(ones_f[0:1, 0:16]),
                            rr(maskb[0:1, b * S:(b + 1) * S]),
                            start=False, stop=True,
                        )
                        nmax = attp.tile([16, 1], F32, tag="nmax", name=f"nm{blk}_{b}")
                        nc.vector.tensor_reduce(
                            nmax[:], sc[:], AX.X, ALU.max, negate=True
                        )
                        se = attp.tile([16, 1], F32, tag="se", name=f"se{blk}_{b}")
                        al = attp.tile([16, S], F32, tag="al", name=f"al{blk}_{b}")
                        nc.scalar.activation(
                            al[:], sc[:], AF.Exp, bias=nmax[:, 0:1],
                            accum_out=se[:, 0:1],
                        )
                        rec = attp.tile([16, 1], F32, tag="rec", name=f"rc{blk}_{b}")
                        nc.vector.reciprocal(rec[:], se[:])
                        aln = attp.tile([16, S], F32, tag="aln", name=f"an{blk}_{b}")
                        nc.vector.tensor_scalar_mul(aln[:], al[:], rec[:, 0:1])
                        alT_ps = ps_a.tile([S, 16], F32, tag="A", name=f"tp{blk}_{b}")
                        nc.tensor.transpose(alT_ps[:], aln[:], ident[0:16, 0:16])
                        alT = attp.tile([S, 16], F32, tag="alT", name=f"at{blk}_{b}")
                        nc.vector.tensor_copy(rr(alT[:]), alT_ps[:])
                        for q in range(4):
                            cx = ps_a.tile([128, 16], F32, tag="A", name=f"cx{blk}_{b}_{q}")
                            nc.tensor.matmul(
                                cx[:],
                                rr(encS[0:S, b * H + q * 128: b * H + (q + 1) * 128]),
                                rr(alT[:]),
                                start=True, stop=True,
                            )
                            nc.vector.tensor_copy(rr(ctxT[:, c0 + q * 4 + b:c1:16]), cx[:])
                    for mh in range(4):
                        hps = ps_a.tile([128, 64], F32, tag="A", name=f"hp{blk}_{mh}")
                        for kt in range(8):
                            srcT = ctxT if kt < 4 else hnewT
                            q = kt % 4
                            rhs = srcT[:].rearrange("p (t x) -> p t x", x=16)[
                                :, blk * 16:(blk + 1) * 16, q * 4:(q + 1) * 4
                            ]
                            nc.tensor.matmul(
                                hps[:],
                                rr(wcc[kt][:, mh * 128:(mh + 1) * 128]),
                                rr(rhs),
                                start=(kt == 0), stop=(kt == 7),
                            )
                        nc.scalar.activation(
                            hot[mh][:, blk * 64:(blk + 1) * 64], hps[:],
                            AF.Tanh, bias=bcc[:, mh:mh + 1]
                        )

        # ---- P5: logits, exp-accum, log-softmax, output ----
        with (
            tc.tile_pool(name="logits", bufs=1) as lgp,
            tc.tile_pool(name="edump", bufs=2) as edp,
            tc.tile_pool(name="sums", bufs=2) as sump,
            tc.tile_pool(name="ostage", bufs=2) as osp,
            tc.tile_pool(name="ps_out", bufs=4, space="PSUM") as ps_out,
        ):
            lts = [lgp.tile([128, V], BF16, tag=f"lt{m}", name=f"lt{m}") for m in range(2)]
            sets = [sump.tile([128, NVCH], F32, tag=f"se{m}", name=f"sums{m}") for m in range(2)]
            for j2 in range(NVCH // 2):
                wch = []
                for q in range(4):
                    w = wop.tile([128, 2 * VCHUNK], BF16, tag="wo")
                    eng = nc.gpsimd if q % 2 == 0 else nc.sync
                    eng.dma_start(
                        w[:], woT_d[q * 128:(q + 1) * 128,
                                    j2 * 2 * VCHUNK:(j2 + 1) * 2 * VCHUNK]
                    )
                    wch.append(w)
                bchk = bop.tile([1, 2 * VCHUNK], BF16, tag="bo")
                nc.gpsimd.dma_start(
                    bchk[:], bout_d[0:1, j2 * 2 * VCHUNK:(j2 + 1) * 2 * VCHUNK]
                )
                for half in range(2):
                    j = j2 * 2 + half
                    hs = slice(half * VCHUNK, (half + 1) * VCHUNK)
                    for m in range(2):
                        ps = ps_out.tile([128, VCHUNK], F32, tag="O")
                        for q in range(4):
                            nc.tensor.matmul(
                                ps[:], hot[q][:, m * 128:(m + 1) * 128], wch[q][:, hs],
                                start=(q == 0), stop=False,
                            )
                        nc.tensor.matmul(
                            ps[:], ones_b[0:1, :], bchk[0:1, hs], start=False, stop=True
                        )
                        dump = edp.tile([128, VCHUNK], F32, tag="edump")
                        nc.scalar.activation(
                            dump[:], ps[:], AF.Exp, accum_out=sets[m][:, j:j + 1]
                        )
                        nc.vector.tensor_copy(
                            lts[m][:, j * VCHUNK:(j + 1) * VCHUNK], ps[:]
                        )
            for m in range(2):
                stot = sump.tile([128, 1], F32, tag="stot")
                nc.vector.tensor_reduce(stot[:], sets[m][:], AX.X, ALU.add)
                lse = sump.tile([128, 1], F32, tag="lse")
                nc.scalar.activation(lse[:], stot[:], AF.Ln)
                for g in range(V // OCHUNK):
                    ost = osp.tile([128, OCHUNK], F32, tag="ost")
                    nc.vector.tensor_scalar_sub(
                        ost[:], lts[m][:, g * OCHUNK:(g + 1) * OCHUNK], lse[:, 0:1]
                    )
                    nc.sync.dma_start(
                        out_d[m * 128:(m + 1) * 128, g * OCHUNK:(g + 1) * OCHUNK],
                        ost[:],
                    )

    nc.compile()
    return nc


_NC_CACHE = None


def _get_program():
    global _NC_CACHE
    if _NC_CACHE is None:
        _NC_CACHE = build_program()
    return _NC_CACHE


def make_core_inputs(all_encoder_hidden_states, initial_decoder_hidden_state,
                     encoder_output_mask, target_input, fra_length, embedding,
                     W_ih, W_hh, b_ih, b_hh, W_cc, b_cc, W_out, b_out):
    """Build the per-core input maps (host-side sharding/layout only)."""
    enc = np.ascontiguousarray(np.asarray(all_encoder_hidden_states, np.float32))
    h0 = np.asarray(initial_decoder_hidden_state, np.float32)[0]
    mask = np.asarray(encoder_output_mask)
    tgt = np.asarray(target_input).astype(np.int64)
    fra = np.asarray(fra_length).astype(np.int64)
    emb = np.ascontiguousarray(np.asarray(embedding, np.float32))
    wihT = np.ascontiguousarray(np.asarray(W_ih, np.float32).T)
    whhT = np.ascontiguousarray(np.asarray(W_hh, np.float32).T)
    bihh = np.ascontiguousarray(
        (np.asarray(b_ih, np.float32) + np.asarray(b_hh, np.float32))[None, :]
    )
    wccT = np.ascontiguousarray(np.asarray(W_cc, np.float32).T)
    bcc4 = np.ascontiguousarray(np.asarray(b_cc, np.float32).reshape(4, 128).T)
    woT = np.ascontiguousarray(
        np.asarray(W_out, np.float32).T.astype(ml_dtypes.bfloat16)
    )
    bout = np.ascontiguousarray(
        np.asarray(b_out, np.float32)[None, :].astype(ml_dtypes.bfloat16)
    )

    in_maps = []
    for c in range(NC):
        bs = slice(c * BL, (c + 1) * BL)
        enc_c = enc[bs]                                   # [BL, S, H]
        ids = tgt[bs].T.reshape(R).astype(np.int32)       # r = t*BL + b
        in_maps.append({
            "emb": emb,
            "ids": np.ascontiguousarray(ids.reshape(2, 128, 1)),
            "h0": np.ascontiguousarray(h0[bs]),
            "encT": np.ascontiguousarray(
                enc_c.transpose(2, 0, 1).reshape(H, BL * S)
            ),
            "encS": np.ascontiguousarray(
                enc_c.transpose(1, 0, 2).reshape(S, BL * H)
            ),
            "maskb": np.ascontiguousarray(
                np.where(mask[bs], 0.0, NEG).astype(np.float32).reshape(1, BL * S)
            ),
            "actm": np.ascontiguousarray(
                (np.arange(T)[None, :] < fra[bs][:, None]).astype(np.float32)
            ),
            "wihT": wihT,
            "onesd": np.ones((1, 128), np.float32),
            "whhT": whhT,
            "bihh": bihh,
            "wccT": wccT,
            "bcc": bcc4,
            "woT": woT,
            "bout": bout,
        })
    return in_maps


def assemble_output(core_outs):
    """core_outs: list of 8 arrays [R, V] fp32 (rows r = t*BL + b)."""
    out = np.empty((B, T, V), np.float32)
    for c in range(NC):
        o = np.asarray(core_outs[c], np.float32).reshape(T, BL, V)
        out[c * BL:(c + 1) * BL] = o.transpose(1, 0, 2)
    return out


def kernel(**inputs) -> np.ndarray:
    from concourse.bass_utils import run_bass_kernel_spmd
    nc = _get_program()
    in_maps = make_core_inputs(**inputs)
    res = run_bass_kernel_spmd(nc, in_maps, list(range(NC)))
    return assemble_output([res.results[c]["out"] for c in range(NC)])
